# revision 5
# baseline (speedup 1.0000x reference)
"""Trainium2 Bass kernel for nn_GAT_15547781612261.

3-layer GATConv (6 heads, concat=False) over an 8192-node / 40960-edge graph
(incl. self loops), with residual, returning final[ptr[1:]-1] -> [8, 1028].

Strategy: only the 8 output rows are needed, so the computation is exactly the
3-hop in-neighborhood of those rows (~500 nodes / ~650 edges at layer 1).  The
host does the integer-only graph slicing and builds 0/1 routing matrices; the
device performs every floating-point operation:

  * per-edge features h_g = x[src_e] @ [W | W@a_src | W@a_dst]  (one matmul)
  * per-dst attention terms gathered via routing matmuls (Gself / ZdstTu)
  * leaky-relu -> clamp -> exp on the edge logits (segment softmax without
    max-subtraction; exact because softmax is shift-invariant and the clamp
    at 80 never binds for sane data)
  * segment sums (softmax denominator and message aggregation) via matmuls
    against the 0/1 dst-routing matrix, with all 6 heads accumulated into the
    same PSUM bank so the head-mean is free

All 8 NeuronCores run the identical program (the pruned problem is far below
one core's roofline; replication avoids collective latency).  Core 0's output
is returned.
"""

import numpy as np

P = 128
H = 6
N_NODES = 8192
CORES = 8

# test harness hooks
TRACE = False
LAST_RESULT = None


def _pad(n, m=P):
    return ((n + m - 1) // m) * m


# ----------------------------------------------------------------------------
# host-side graph slicing (integer work only)
# ----------------------------------------------------------------------------

def _slice_layer(dst_unique, src_all, dst_all):
    """Edges into dst_unique; local indices; self-loop edge of each dst."""
    mask = np.isin(dst_all, dst_unique)
    e_src = src_all[mask]
    e_dst = dst_all[mask]
    src_nodes = np.unique(e_src)
    esl = np.searchsorted(src_nodes, e_src)
    edl = np.searchsorted(dst_unique, e_dst)
    order = np.argsort(edl, kind="stable")
    esl, edl = esl[order], edl[order]
    is_self = e_src[order] == e_dst[order]
    self_edge = np.full(len(dst_unique), -1, np.int64)
    for e_i in np.flatnonzero(is_self):
        if self_edge[edl[e_i]] < 0:
            self_edge[edl[e_i]] = e_i
    assert (self_edge >= 0).all(), "self loop missing for some dst"
    return src_nodes, esl, edl, self_edge


def _routing(esl, edl, self_edge, n_src, n_dst, agg_cols=None):
    """Build 0/1 routing matrices (fp32) for one layer."""
    E = len(esl)
    Ep = _pad(E)
    Sp = _pad(n_src)
    Dup = _pad(n_dst)
    Zdst = np.zeros((Ep, n_dst), np.float32)
    Zdst[np.arange(E), edl] = 1.0
    ZdstTu = np.zeros((Dup, Ep), np.float32)
    ZdstTu[edl, np.arange(E)] = 1.0
    Gself = np.zeros((Ep, n_dst), np.float32)
    Gself[self_edge, np.arange(n_dst)] = 1.0
    Gsrc = np.zeros((Sp, Ep), np.float32)
    Gsrc[esl, np.arange(E)] = 1.0
    if agg_cols is None:
        Zagg = Zdst
        n_agg = n_dst
    else:
        n_agg = len(agg_cols)
        Zagg = np.zeros((Ep, n_agg), np.float32)
        for col, d in enumerate(agg_cols):
            Zagg[np.arange(E)[edl == d], col] = 1.0
    return dict(E=E, Ep=Ep, Sp=Sp, Du=n_dst, Dup=Dup, n_agg=n_agg,
                Zdst=Zdst, ZdstTu=ZdstTu, Gself=Gself, Gsrc=Gsrc, Zagg=Zagg)


def _fold_weights(W, a_src, a_dst, cinp):
    """[W | W_k @ as_k | W_k @ ad_k], zero-padded to cinp rows."""
    W = np.asarray(W, np.float32)
    a_src = np.asarray(a_src, np.float32)
    a_dst = np.asarray(a_dst, np.float32)
    Cin = W.shape[0]
    C = a_src.shape[1]
    Wh = W.reshape(Cin, H, C)
    Was = np.einsum('ihc,hc->ih', Wh, a_src)
    Wad = np.einsum('ihc,hc->ih', Wh, a_dst)
    Waug = np.concatenate([W, Was, Wad], axis=1)
    out = np.zeros((cinp, Waug.shape[1]), np.float32)
    out[:Cin] = Waug
    return np.ascontiguousarray(out)


def _host_prep(x, edge_index, ptr, params):
    x = np.ascontiguousarray(np.asarray(x, np.float32))
    ei = np.asarray(edge_index, np.int64)
    ptr = np.asarray(ptr, np.int64)
    loops = np.arange(N_NODES, dtype=np.int64)
    src_all = np.concatenate([ei[0], loops])
    dst_all = np.concatenate([ei[1], loops])
    R = (ptr[1:] - 1) % N_NODES

    D3u = np.unique(R)
    S3, es3, ed3, se3 = _slice_layer(D3u, src_all, dst_all)
    S2, es2, ed2, se2 = _slice_layer(S3, src_all, dst_all)
    S1, es1, ed1, se1 = _slice_layer(S2, src_all, dst_all)

    l3 = _routing(es3, ed3, se3, len(S3), len(D3u),
                  agg_cols=np.searchsorted(D3u, R))
    l2 = _routing(es2, ed2, se2, len(S2), len(S3))
    l1 = _routing(es1, ed1, se1, len(S1), len(S2))

    dims = [x.shape[1]] + [params[f'as{i}'].shape[1] for i in (1, 2, 3)]
    cinp = [_pad(d) for d in dims]

    # layer-1 edge-major routed input: XE1T[:, e] = x[src_global(e)]
    gsrc1 = S1[es1]                       # global src per edge
    XE1T = np.zeros((cinp[0], l1["Ep"]), np.float32)
    XE1T[:dims[0], :l1["E"]] = x[gsrc1].T

    consts = {
        "XE1T": XE1T,
        "XR": np.ascontiguousarray(x[R]),
    }
    for li, (lay, cp) in enumerate(zip((l1, l2, l3), cinp[:3]), 1):
        consts[f"W{li}"] = _fold_weights(params[f'W{li}'], params[f'as{li}'],
                                         params[f'ad{li}'], cp)
        b = np.asarray(params[f'b{li}'], np.float32)
        rows = 8 if li == 3 else P
        consts[f"B{li}"] = np.ascontiguousarray(
            np.broadcast_to(b[None, :], (rows, len(b))).copy())
        consts[f"Zdst{li}"] = lay["Zdst"]
        consts[f"ZdstTu{li}"] = lay["ZdstTu"]
        consts[f"Gself{li}"] = lay["Gself"]
        if li > 1:
            consts[f"Gsrc{li}"] = lay["Gsrc"]
        if li == 3:
            consts[f"Zagg{li}"] = lay["Zagg"]
    consts = {k: np.ascontiguousarray(v, dtype=np.float32)
              for k, v in consts.items()}
    return consts, (l1, l2, l3), dims


# ----------------------------------------------------------------------------
# device program
# ----------------------------------------------------------------------------

def _nchunks(total, step):
    out = []
    o = 0
    while o < total:
        out.append((o, min(o + step, total)))
        o += step
    return out


def _build_program(layers, dims):
    import concourse.bacc as bacc
    import concourse.tile as tile
    from concourse import mybir
    from concourse.masks import make_identity

    f32 = mybir.dt.float32
    Alu = mybir.AluOpType
    Act = mybir.ActivationFunctionType

    l1, l2, l3 = layers
    slopes = [0.2, 0.2, 0.0]
    C_out = [dims[1], dims[2], dims[3]]

    nc = bacc.Bacc("TRN2", target_bir_lowering=False)

    din = {}
    for li, lay in enumerate((l1, l2, l3), 1):
        C = C_out[li - 1]
        HCw = H * C + 2 * H
        cinp = _pad(dims[li - 1])
        din[f"W{li}"] = nc.dram_tensor(f"W{li}", [cinp, HCw], f32,
                                       kind="ExternalInput")
        din[f"Zdst{li}"] = nc.dram_tensor(f"Zdst{li}", [lay["Ep"], lay["Du"]],
                                          f32, kind="ExternalInput")
        din[f"ZdstTu{li}"] = nc.dram_tensor(f"ZdstTu{li}",
                                            [lay["Dup"], lay["Ep"]], f32,
                                            kind="ExternalInput")
        din[f"Gself{li}"] = nc.dram_tensor(f"Gself{li}", [lay["Ep"], lay["Du"]],
                                           f32, kind="ExternalInput")
        if li > 1:
            din[f"Gsrc{li}"] = nc.dram_tensor(f"Gsrc{li}",
                                              [lay["Sp"], lay["Ep"]], f32,
                                              kind="ExternalInput")
        if li == 3:
            din[f"Zagg{li}"] = nc.dram_tensor(f"Zagg{li}",
                                              [lay["Ep"], lay["n_agg"]], f32,
                                              kind="ExternalInput")
        rows = 8 if li == 3 else P
        din[f"B{li}"] = nc.dram_tensor(f"B{li}", [rows, C], f32,
                                       kind="ExternalInput")
    din["XE1T"] = nc.dram_tensor("XE1T", [_pad(dims[0]), l1["Ep"]], f32,
                                 kind="ExternalInput")
    din["XR"] = nc.dram_tensor("XR", [8, dims[3]], f32, kind="ExternalInput")
    dout = nc.dram_tensor("out", [8, dims[3]], f32, kind="ExternalOutput")

    def gat_layer(pools, li, lay, XET_tiles, out_rows_per_chunk, out_tiles):
        """Emit one GAT layer.  XET_tiles: list of [128, Ep] sbuf tiles
        (K-tiles of edge-major transposed input).  Writes node-major
        (mean-over-heads + bias) output into out_tiles ([128, C] each,
        rows given by out_rows_per_chunk)."""
        work, psum = pools
        C = C_out[li - 1]
        HC = H * C
        HCw = HC + 2 * H
        Ep, Du, Dup, n_agg = lay["Ep"], lay["Du"], lay["Dup"], lay["n_agg"]
        nE = Ep // P
        nK = len(XET_tiles)
        nDup = Dup // P
        slope = slopes[li - 1]

        # ---- load constants
        W_t = []
        for k in range(nK):
            t = work.tile([P, HCw], f32, name=f"W{li}_{k}", tag=f"W{li}_{k}")
            nc.sync.dma_start(out=t[:], in_=din[f"W{li}"][k * P:(k + 1) * P, :])
            W_t.append(t)
        Zdst_t, Gself_t, ZdstTu_t = [], [], []
        for e in range(nE):
            t = work.tile([P, Du], f32, name=f"Zdst{li}_{e}", tag=f"Zdst{li}_{e}")
            nc.sync.dma_start(out=t[:], in_=din[f"Zdst{li}"][e * P:(e + 1) * P, :])
            Zdst_t.append(t)
            t = work.tile([P, Du], f32, name=f"Gself{li}_{e}", tag=f"Gself{li}_{e}")
            nc.sync.dma_start(out=t[:], in_=din[f"Gself{li}"][e * P:(e + 1) * P, :])
            Gself_t.append(t)
        for d in range(nDup):
            t = work.tile([P, Ep], f32, name=f"ZdstTu{li}_{d}", tag=f"ZdstTu{li}_{d}")
            nc.sync.dma_start(out=t[:], in_=din[f"ZdstTu{li}"][d * P:(d + 1) * P, :])
            ZdstTu_t.append(t)
        if li == 3:
            Zagg_t = []
            for e in range(nE):
                t = work.tile([P, n_agg], f32, name=f"Zagg{li}_{e}", tag=f"Zagg{li}_{e}")
                nc.sync.dma_start(out=t[:], in_=din[f"Zagg{li}"][e * P:(e + 1) * P, :])
                Zagg_t.append(t)
        else:
            Zagg_t = Zdst_t
        bias_rows = 8 if li == 3 else P
        bias_t = work.tile([bias_rows, C], f32, name=f"B{li}", tag=f"B{li}")
        nc.sync.dma_start(out=bias_t[:], in_=din[f"B{li}"][:, :])

        # ---- 1. per-edge features h_g = XE^T.T @ W_aug   [Ep, HCw]
        h_t = []
        for e in range(nE):
            t = work.tile([P, HCw], f32, name=f"hg{li}_{e}", tag=f"hg{li}_{e}")
            h_t.append(t)
            for (n0, n1) in _nchunks(HCw, 512):
                ps = psum.tile([P, n1 - n0], f32, name="ps_h", tag="ps_h", bufs=2)
                for k in range(nK):
                    nc.tensor.matmul(
                        out=ps[:],
                        lhsT=XET_tiles[k][:, e * P:(e + 1) * P],
                        rhs=W_t[k][:, n0:n1],
                        start=(k == 0), stop=(k == nK - 1))
                nc.vector.tensor_copy(out=t[:, n0:n1], in_=ps[:])

        # ---- 2. ed at dst nodes: ed_node[d] = h_g[self_edge(d), ed-cols]
        edn_t = []
        for d in range(nDup):
            t = work.tile([P, H], f32, name=f"edn{li}_{d}", tag=f"edn{li}_{d}")
            nc.vector.memset(t[:], 0.0)
            edn_t.append(t)
        for (d0, d1) in _nchunks(Du, P):
            dc = d0 // P
            rows = d1 - d0
            ps = psum.tile([P, H], f32, name="ps_edn", tag="ps_small", bufs=2)
            for e in range(nE):
                nc.tensor.matmul(
                    out=ps[:rows, :],
                    lhsT=Gself_t[e][:, d0:d1],
                    rhs=h_t[e][:, HC + H:HC + 2 * H],
                    start=(e == 0), stop=(e == nE - 1))
            nc.vector.tensor_copy(out=edn_t[dc][:rows, :], in_=ps[:rows, :])

        # ---- 3. per-edge logits -> ex = exp(clamp(lrelu(es + ed_g)))
        ex_t = []
        for e in range(nE):
            ps = psum.tile([P, H], f32, name="ps_edg", tag="ps_small", bufs=2)
            for d in range(nDup):
                nc.tensor.matmul(
                    out=ps[:],
                    lhsT=ZdstTu_t[d][:, e * P:(e + 1) * P],
                    rhs=edn_t[d][:],
                    start=(d == 0), stop=(d == nDup - 1))
            t = work.tile([P, H], f32, name=f"ex{li}_{e}", tag=f"ex{li}_{e}")
            ex_t.append(t)
            nc.vector.tensor_tensor(out=t[:], in0=h_t[e][:, HC:HC + H],
                                    in1=ps[:], op=Alu.add)
            # leaky relu: max(x, slope*x)  (slope in [0, 1])
            nc.vector.scalar_tensor_tensor(out=t[:], in0=t[:],
                                           scalar=float(slope), in1=t[:],
                                           op0=Alu.mult, op1=Alu.max)
            nc.vector.tensor_scalar_min(out=t[:], in0=t[:], scalar1=80.0)
            nc.scalar.activation(out=t[:], in_=t[:], func=Act.Exp)

        # ---- 4. softmax denominators z[h, d] then rz = 1/max(z, tiny)
        zp = psum.tile([H, Du], f32, name="ps_z", tag="ps_small", bufs=2)
        for e in range(nE):
            nc.tensor.matmul(out=zp[:], lhsT=ex_t[e][:], rhs=Zdst_t[e][:, :],
                             start=(e == 0), stop=(e == nE - 1))
        rz = work.tile([H, Du], f32, name=f"rz{li}", tag=f"rz{li}")
        nc.vector.tensor_scalar_max(out=rz[:], in0=zp[:], scalar1=1e-30)
        nc.vector.reciprocal(out=rz[:], in_=rz[:])

        # ---- 5. rz transposed to node-major [Dup, H]
        rzT_t = []
        for d in range(nDup):
            t = work.tile([P, H], f32, name=f"rzT{li}_{d}", tag=f"rzT{li}_{d}")
            nc.vector.memset(t[:], 0.0)
            rzT_t.append(t)
        for (d0, d1) in _nchunks(Du, P):
            cols = d1 - d0
            ps = psum.tile([P, H], f32, name="ps_rzT", tag="ps_small", bufs=2)
            nc.tensor.transpose(out=ps[:cols, :], in_=rz[:, d0:d1],
                                identity=ident[:H, :H])
            nc.vector.tensor_copy(out=rzT_t[d0 // P][:cols, :], in_=ps[:cols, :])

        # ---- 6. alpha = ex * rz[dst_e]
        al_t = []
        for e in range(nE):
            ps = psum.tile([P, H], f32, name="ps_rzg", tag="ps_small", bufs=2)
            for d in range(nDup):
                nc.tensor.matmul(
                    out=ps[:],
                    lhsT=ZdstTu_t[d][:, e * P:(e + 1) * P],
                    rhs=rzT_t[d][:],
                    start=(d == 0), stop=(d == nDup - 1))
            t = work.tile([P, H], f32, name=f"al{li}_{e}", tag=f"al{li}_{e}")
            al_t.append(t)
            nc.vector.tensor_tensor(out=t[:], in0=ex_t[e][:], in1=ps[:],
                                    op=Alu.mult)

        # ---- 7. aggregation: out[d] = (1/H) sum_k sum_e alpha Zagg h + b
        for (a0, a1) in _nchunks(n_agg, P):
            ac = a0 // P
            rows = a1 - a0
            cch = _nchunks(C, 512)
            ps_list = [psum.tile([P, c1 - c0], f32, name=f"ps_agg{i}", tag=f"ps_agg{i}", bufs=1)
                       for i, (c0, c1) in enumerate(cch)]
            for k in range(H):
                for e in range(nE):
                    za = work.tile([P, rows], f32, name="za", tag="za", bufs=3)
                    nc.vector.tensor_scalar_mul(out=za[:, :],
                                                in0=Zagg_t[e][:, a0:a1],
                                                scalar1=al_t[e][:, k:k + 1])
                    first = (k == 0 and e == 0)
                    last = (k == H - 1 and e == nE - 1)
                    for (c0, c1), ps in zip(cch, ps_list):
                        nc.tensor.matmul(out=ps[:rows, :], lhsT=za[:, :],
                                         rhs=h_t[e][:, k * C + c0:k * C + c1],
                                         start=first, stop=last)
            for (c0, c1), ps in zip(cch, ps_list):
                nc.vector.scalar_tensor_tensor(
                    out=out_tiles[ac][:rows, c0:c1],
                    in0=ps[:rows, :], scalar=1.0 / H,
                    in1=bias_t[:rows, c0:c1] if bias_rows == P
                        else bias_t[:rows, c0:c1],
                    op0=Alu.mult, op1=Alu.add)

    def xe_gather(pools, li, lay, X_tiles, Cprev):
        """XE^T [Cprev-tiles of 128, Ep] = X^T routed to edges via Gsrc."""
        work, psum = pools
        Ep, Sp = lay["Ep"], lay["Sp"]
        nS = Sp // P
        Gsrc_t = []
        for s in range(nS):
            t = work.tile([P, Ep], f32, name=f"Gsrc{li}_{s}", tag=f"Gsrc{li}_{s}")
            nc.sync.dma_start(out=t[:], in_=din[f"Gsrc{li}"][s * P:(s + 1) * P, :])
            Gsrc_t.append(t)
        XET = []
        for m in range(Cprev // P):
            ps = psum.tile([P, Ep], f32, name="ps_xe", tag="ps_small", bufs=2)
            for s in range(nS):
                nc.tensor.matmul(out=ps[:],
                                 lhsT=X_tiles[s][:, m * P:(m + 1) * P],
                                 rhs=Gsrc_t[s][:],
                                 start=(s == 0), stop=(s == nS - 1))
            t = work.tile([P, Ep], f32, name=f"XET{li}_{m}", tag=f"XET{li}_{m}")
            nc.vector.tensor_copy(out=t[:], in_=ps[:])
            XET.append(t)
        return XET

    with tile.TileContext(nc) as tc:
        with tc.tile_pool(name="carry", bufs=1) as carry, \
             tc.tile_pool(name="psum", bufs=1, space="PSUM") as psum:
            ident = carry.tile([P, P], f32, name="ident", tag="ident")
            make_identity(nc, ident[:])

            # carried node-major activations
            nX2 = l2["Sp"] // P
            X2_t = [carry.tile([P, C_out[0]], f32, name=f"X2_{i}", tag=f"X2_{i}")
                    for i in range(nX2)]
            nX3 = l3["Sp"] // P
            X3_t = [carry.tile([P, C_out[1]], f32, name=f"X3_{i}", tag=f"X3_{i}")
                    for i in range(nX3)]
            for t in X2_t:
                nc.vector.memset(t[:], 0.0)
            for t in X3_t:
                nc.vector.memset(t[:], 0.0)

            # ---------------- layer 1
            with tc.tile_pool(name="l1", bufs=1) as w1:
                cinp1 = _pad(dims[0])
                XE1T_t = []
                for k in range(cinp1 // P):
                    t = w1.tile([P, l1["Ep"]], f32, name=f"XE1T_{k}", tag=f"XE1T_{k}")
                    nc.sync.dma_start(out=t[:],
                                      in_=din["XE1T"][k * P:(k + 1) * P, :])
                    XE1T_t.append(t)
                rows_per_chunk = [d1 - d0 for d0, d1 in _nchunks(l1["n_agg"], P)]
                gat_layer((w1, psum), 1, l1, XE1T_t, rows_per_chunk, X2_t)

            # ---------------- layer 2
            with tc.tile_pool(name="l2", bufs=1) as w2:
                XE2T_t = xe_gather((w2, psum), 2, l2, X2_t, _pad(C_out[0]))
                rows_per_chunk = [d1 - d0 for d0, d1 in _nchunks(l2["n_agg"], P)]
                gat_layer((w2, psum), 2, l2, XE2T_t, rows_per_chunk, X3_t)

            # ---------------- layer 3 (+ residual, output)
            with tc.tile_pool(name="l3", bufs=1) as w3:
                XE3T_t = xe_gather((w3, psum), 3, l3, X3_t, _pad(C_out[1]))
                out_f = w3.tile([8, dims[3]], f32, name="out_f", tag="out_f")
                rows_per_chunk = [8]
                gat_layer((w3, psum), 3, l3, XE3T_t, rows_per_chunk, [out_f])
                xr_t = w3.tile([8, dims[3]], f32, name="XR", tag="XR")
                nc.sync.dma_start(out=xr_t[:], in_=din["XR"][:, :])
                nc.vector.tensor_tensor(out=out_f[:], in0=out_f[:],
                                        in1=xr_t[:], op=Alu.add)
                nc.sync.dma_start(out=dout[:, :], in_=out_f[:])

    nc.finalize()
    return nc


def kernel(**inputs):
    global LAST_RESULT
    x = inputs["x"]
    edge_index = inputs["edge_index"]
    ptr = inputs["ptr"]
    consts, layers, dims = _host_prep(x, edge_index, ptr, inputs)
    nc = _build_program(layers, dims)

    from concourse.bass_utils import run_bass_kernel_spmd
    in_maps = [consts for _ in range(CORES)]
    res = run_bass_kernel_spmd(nc, in_maps, list(range(CORES)), trace=TRACE)
    LAST_RESULT = res
    return np.asarray(res.results[0]["out"], np.float32)


# revision 9
# speedup vs baseline: 1.9731x; 1.9731x over previous
"""Trainium2 Bass kernel for nn_GAT_15547781612261.

3-layer GATConv (6 heads, concat=False) over an 8192-node / 40960-edge graph
(incl. self loops), with residual, returning final[ptr[1:]-1] -> [8, 1028].

Strategy: only the 8 output rows are needed, so the computation is exactly the
3-hop in-neighborhood of those rows (~500 nodes / ~650 edges at layer 1).  The
host does the integer-only graph slicing and builds 0/1 routing matrices; the
device performs every floating-point operation:

  * per-edge features h_g = x[src_e] @ [W | W@a_src | W@a_dst]  (one matmul)
  * per-dst attention terms gathered via routing matmuls (Gself / ZdstTu)
  * leaky-relu -> clamp -> exp on the edge logits (segment softmax without
    max-subtraction; exact because softmax is shift-invariant and the clamp
    at 80 never binds for sane data)
  * segment sums (softmax denominator and message aggregation) via matmuls
    against the 0/1 dst-routing matrix, with heads accumulated into the
    same PSUM banks so the head-mean is nearly free

Precision split: the bulky matmuls (features, messages) run in float32r
(fast 1-cycle/row PE mode, ~1e-4 rounding); the softmax statistics path
(ed/z/rz gathers) stays in exact fp32 so attention ratios keep full
precision.  Constants are packed into six [128, N] images so each loads
with a single large DMA.

All 8 NeuronCores run the identical program (the pruned problem is far below
one core's roofline; replication avoids collective latency).  Core 0's output
is returned.
"""

import numpy as np

P = 128
H = 6
N_NODES = 8192
CORES = 8

# test harness hooks
TRACE = False
LAST_RESULT = None


def _pad(n, m=P):
    return ((n + m - 1) // m) * m


# ----------------------------------------------------------------------------
# host-side graph slicing (integer work only)
# ----------------------------------------------------------------------------

def _slice_layer(dst_unique, src_all, dst_all):
    """Edges into dst_unique; local indices; self-loop edge of each dst."""
    mask = np.isin(dst_all, dst_unique)
    e_src = src_all[mask]
    e_dst = dst_all[mask]
    src_nodes = np.unique(e_src)
    esl = np.searchsorted(src_nodes, e_src)
    edl = np.searchsorted(dst_unique, e_dst)
    order = np.argsort(edl, kind="stable")
    esl, edl = esl[order], edl[order]
    is_self = e_src[order] == e_dst[order]
    self_edge = np.full(len(dst_unique), -1, np.int64)
    for e_i in np.flatnonzero(is_self):
        if self_edge[edl[e_i]] < 0:
            self_edge[edl[e_i]] = e_i
    assert (self_edge >= 0).all(), "self loop missing for some dst"
    return src_nodes, esl, edl, self_edge


def _routing(esl, edl, self_edge, n_src, n_dst, agg_cols=None):
    """Build 0/1 routing matrices (fp32) for one layer."""
    E = len(esl)
    Ep = _pad(E)
    Sp = _pad(n_src)
    Dup = _pad(n_dst)
    Zdst = np.zeros((Ep, Dup), np.float32)
    Zdst[np.arange(E), edl] = 1.0
    ZdstTu = np.zeros((Dup, Ep), np.float32)
    ZdstTu[edl, np.arange(E)] = 1.0
    Gself = np.zeros((Ep, Dup), np.float32)
    Gself[self_edge, np.arange(n_dst)] = 1.0
    Gsrc = np.zeros((Sp, Ep), np.float32)
    Gsrc[esl, np.arange(E)] = 1.0
    if agg_cols is None:
        Zagg = Zdst
        n_agg = n_dst
    else:
        n_agg = len(agg_cols)
        Zagg = np.zeros((Ep, n_agg), np.float32)
        for col, d in enumerate(agg_cols):
            Zagg[np.arange(E)[edl == d], col] = 1.0
    return dict(E=E, Ep=Ep, Sp=Sp, Du=n_dst, Dup=Dup, n_agg=n_agg,
                Zdst=Zdst, ZdstTu=ZdstTu, Gself=Gself, Gsrc=Gsrc, Zagg=Zagg)


def _fold_weights(W, a_src, a_dst, cinp):
    """[W | W_k @ as_k | W_k @ ad_k], zero-padded to cinp rows."""
    W = np.asarray(W, np.float32)
    a_src = np.asarray(a_src, np.float32)
    a_dst = np.asarray(a_dst, np.float32)
    Cin = W.shape[0]
    C = a_src.shape[1]
    Wh = W.reshape(Cin, H, C)
    Was = np.einsum('ihc,hc->ih', Wh, a_src)
    Wad = np.einsum('ihc,hc->ih', Wh, a_dst)
    Waug = np.concatenate([W, Was, Wad], axis=1)
    out = np.zeros((cinp, Waug.shape[1]), np.float32)
    out[:Cin] = Waug
    return np.ascontiguousarray(out)


class _Pack:
    """Stacks [t*128, C] (or [rows<=128, C]) fp32 arrays into one [128, N]
    image loaded with a single DMA; records per-block column offsets."""

    def __init__(self, name):
        self.name = name
        self.cols = 0
        self.blocks = {}     # key -> (offset, block_cols, n_tiles)
        self.chunks = []

    def add(self, key, arr):
        r, c = arr.shape
        if r <= P:
            tiles = [np.vstack([arr, np.zeros((P - r, c), np.float32)])
                     if r < P else arr]
        else:
            assert r % P == 0
            tiles = [arr[i * P:(i + 1) * P] for i in range(r // P)]
        self.blocks[key] = (self.cols, c, len(tiles))
        for t in tiles:
            self.chunks.append(np.ascontiguousarray(t, np.float32))
            self.cols += c

    def image(self):
        return np.ascontiguousarray(np.concatenate(self.chunks, axis=1))


def _host_prep(x, edge_index, ptr, params):
    x = np.ascontiguousarray(np.asarray(x, np.float32))
    ei = np.asarray(edge_index, np.int64)
    ptr = np.asarray(ptr, np.int64)
    loops = np.arange(N_NODES, dtype=np.int64)
    src_all = np.concatenate([ei[0], loops])
    dst_all = np.concatenate([ei[1], loops])
    R = (ptr[1:] - 1) % N_NODES

    D3u = np.unique(R)
    S3, es3, ed3, se3 = _slice_layer(D3u, src_all, dst_all)
    S2, es2, ed2, se2 = _slice_layer(S3, src_all, dst_all)
    S1, es1, ed1, se1 = _slice_layer(S2, src_all, dst_all)

    l3 = _routing(es3, ed3, se3, len(S3), len(D3u),
                  agg_cols=np.searchsorted(D3u, R))
    l2 = _routing(es2, ed2, se2, len(S2), len(S3))
    l1 = _routing(es1, ed1, se1, len(S1), len(S2))

    dims = [x.shape[1]] + [params[f'as{i}'].shape[1] for i in (1, 2, 3)]

    # layer-1 edge-major routed input: XE1T[:, e] = x[src_global(e)]
    XE1T = np.zeros((_pad(dims[0]), l1["Ep"]), np.float32)
    XE1T[:dims[0], :l1["E"]] = x[S1[es1]].T

    def bias_img(li, rows):
        b = np.asarray(params[f'b{li}'], np.float32)
        return np.ascontiguousarray(
            np.broadcast_to(b[None, :], (rows, len(b))).copy())

    g1r = _Pack("g1r")
    g1r.add("XE1T", XE1T)
    g1r.add("W1", _fold_weights(params['W1'], params['as1'], params['ad1'],
                                _pad(dims[0])))
    g1r.add("Zdst1", l1["Zdst"])
    g1f = _Pack("g1f")
    g1f.add("Gself1", l1["Gself"])
    g1f.add("ZdstTu1", l1["ZdstTu"])
    g1f.add("B1", bias_img(1, P))

    g2r = _Pack("g2r")
    g2r.add("W2", _fold_weights(params['W2'], params['as2'], params['ad2'],
                                _pad(dims[1])))
    g2r.add("Gsrc2", l2["Gsrc"])
    g2r.add("Zdst2", l2["Zdst"])
    g2f = _Pack("g2f")
    g2f.add("Gself2", l2["Gself"])
    g2f.add("ZdstTu2", l2["ZdstTu"])
    g2f.add("B2", bias_img(2, P))

    g3r = _Pack("g3r")
    g3r.add("W3", _fold_weights(params['W3'], params['as3'], params['ad3'],
                                _pad(dims[2])))
    g3r.add("Gsrc3", l3["Gsrc"])
    g3r.add("Zagg3", l3["Zagg"])
    g3f = _Pack("g3f")
    g3f.add("Gself3", l3["Gself"])
    g3f.add("ZdstTu3", l3["ZdstTu"])
    g3f.add("Zdst3", l3["Zdst"])
    g3f.add("B3", bias_img(3, 8))
    g3f.add("XR", np.ascontiguousarray(x[R]))

    packs = dict(g1r=g1r, g1f=g1f, g2r=g2r, g2f=g2f, g3r=g3r, g3f=g3f)
    consts = {nm: p.image() for nm, p in packs.items()}
    return consts, packs, (l1, l2, l3), dims


# ----------------------------------------------------------------------------
# device program
# ----------------------------------------------------------------------------

def _nchunks(total, step):
    out = []
    o = 0
    while o < total:
        out.append((o, min(o + step, total)))
        o += step
    return out


def _build_program(packs, layers, dims):
    import concourse.bacc as bacc
    import concourse.tile as tile
    from concourse import mybir
    from concourse.masks import make_identity

    f32 = mybir.dt.float32
    f32r = mybir.dt.float32r
    Alu = mybir.AluOpType
    Act = mybir.ActivationFunctionType

    l1, l2, l3 = layers
    slopes = [0.2, 0.2, 0.0]
    C_out = [dims[1], dims[2], dims[3]]

    nc = bacc.Bacc("TRN2", target_bir_lowering=False)

    din = {}
    for nm, p in packs.items():
        dt = f32r if nm.endswith("r") else f32
        din[nm] = nc.dram_tensor(nm, [P, p.cols], dt, kind="ExternalInput")
    dout = nc.dram_tensor("out", [8, dims[3]], f32, kind="ExternalOutput")

    # pack sbuf tiles, filled inside the TileContext
    ptile = {}

    def pv(grp, key, t=0, c0=None, c1=None):
        """View of K-tile `t` of block `key` in pack `grp`, cols [c0, c1)."""
        off, c, _ntl = packs[grp].blocks[key]
        lo = off + t * c + (c0 or 0)
        hi = off + t * c + (c1 if c1 is not None else c)
        return ptile[grp][:, lo:hi]

    def gat_layer(pools, li, lay, XET, rg, fg, out_writer):
        """Emit one GAT layer.
        XET: list of [128, Ep] f32r APs (K-tiles of edge-major input).
        rg/fg: pack-group names for f32r / f32 constants.
        out_writer: (dchunk, rows, acc_or_pslist, cch) -> writes node rows."""
        work, psum = pools
        C = C_out[li - 1]
        HC = H * C
        HCw = HC + 2 * H
        Ep, Du, Dup, n_agg = lay["Ep"], lay["Du"], lay["Dup"], lay["n_agg"]
        nE = Ep // P
        nK = len(XET)
        nDt = Dup // P
        slope = slopes[li - 1]

        # ---- 1. per-edge features h_g = XE^T.T @ W_aug   [Ep, HCw] (f32r)
        h_t = []
        for e in range(nE):
            t = work.tile([P, HCw], f32r, name=f"hg{li}_{e}", tag=f"hg{li}_{e}")
            h_t.append(t)
            for (n0, n1) in _nchunks(HCw, 512):
                ps = psum.tile([P, n1 - n0], f32, name="ps_h", tag="ps_h",
                               bufs=2)
                for k in range(nK):
                    nc.tensor.matmul(
                        out=ps[:],
                        lhsT=XET[k][:, e * P:(e + 1) * P],
                        rhs=pv(rg, f"W{li}", k, n0, n1),
                        start=(k == 0), stop=(k == nK - 1))
                nc.vector.tensor_copy(out=t[:, n0:n1], in_=ps[:])

        def hs(e, c0, c1, as_f32=False):
            ap = h_t[e][:, c0:c1]
            return ap.bitcast(f32) if as_f32 else ap

        # ---- 2. ed at dst nodes: ed_node[d] = h_g[self_edge(d), ed-cols]
        edn_t = []
        for d in range(nDt):
            t = work.tile([P, H], f32, name=f"edn{li}_{d}", tag=f"edn{li}_{d}")
            edn_t.append(t)
        for (d0, d1) in _nchunks(Dup, P):
            ps = psum.tile([P, H], f32, name="ps_edn", tag="ps_small", bufs=2)
            for e in range(nE):
                nc.tensor.matmul(
                    out=ps[:],
                    lhsT=pv(fg, f"Gself{li}", e, d0, d1),
                    rhs=hs(e, HC + H, HC + 2 * H, True),
                    start=(e == 0), stop=(e == nE - 1))
            nc.vector.tensor_copy(out=edn_t[d0 // P][:], in_=ps[:])

        # ---- 3. per-edge logits -> ex = exp(clamp(lrelu(es + ed_g)))
        ex_t = []
        for e in range(nE):
            ps = psum.tile([P, H], f32, name="ps_edg", tag="ps_small", bufs=2)
            for d in range(nDt):
                nc.tensor.matmul(
                    out=ps[:],
                    lhsT=pv(fg, f"ZdstTu{li}", d, e * P, (e + 1) * P),
                    rhs=edn_t[d][:],
                    start=(d == 0), stop=(d == nDt - 1))
            t = work.tile([P, H], f32, name=f"ex{li}_{e}", tag=f"ex{li}_{e}")
            ex_t.append(t)
            nc.vector.tensor_tensor(out=t[:], in0=hs(e, HC, HC + H, True),
                                    in1=ps[:], op=Alu.add)
            # leaky relu: max(x, slope*x)  (slope in [0, 1])
            nc.vector.scalar_tensor_tensor(out=t[:], in0=t[:],
                                           scalar=float(slope), in1=t[:],
                                           op0=Alu.mult, op1=Alu.max)
            nc.vector.tensor_scalar_min(out=t[:], in0=t[:], scalar1=80.0)
            nc.scalar.activation(out=t[:], in_=t[:], func=Act.Exp)

        # ---- 4. softmax denominators z[h, d] then rz = 1/max(z, tiny)
        zp = psum.tile([H, Dup], f32, name="ps_z", tag="ps_small", bufs=2)
        for e in range(nE):
            rhs = (pv(fg, "Zdst3", e) if li == 3
                   else pv(rg, f"Zdst{li}", e).bitcast(f32))
            nc.tensor.matmul(out=zp[:], lhsT=ex_t[e][:], rhs=rhs,
                             start=(e == 0), stop=(e == nE - 1))
        rz = work.tile([H, Dup], f32, name=f"rz{li}", tag=f"rz{li}")
        nc.vector.tensor_scalar_max(out=rz[:], in0=zp[:], scalar1=1e-30)
        nc.vector.reciprocal(out=rz[:], in_=rz[:])

        # ---- 5. rz transposed to node-major [Dup, H]
        rzT_t = []
        for d in range(nDt):
            t = work.tile([P, H], f32, name=f"rzT{li}_{d}", tag=f"rzT{li}_{d}")
            rzT_t.append(t)
        for (d0, d1) in _nchunks(Dup, P):
            ps = psum.tile([P, H], f32, name="ps_rzT", tag="ps_small", bufs=2)
            nc.tensor.transpose(out=ps[:], in_=rz[:, d0:d1],
                                identity=ident[:H, :H])
            nc.vector.tensor_copy(out=rzT_t[d0 // P][:], in_=ps[:])

        # ---- 6. alpha = ex * rz[dst_e]
        al_t = []
        for e in range(nE):
            ps = psum.tile([P, H], f32, name="ps_rzg", tag="ps_small", bufs=2)
            for d in range(nDt):
                nc.tensor.matmul(
                    out=ps[:],
                    lhsT=pv(fg, f"ZdstTu{li}", d, e * P, (e + 1) * P),
                    rhs=rzT_t[d][:],
                    start=(d == 0), stop=(d == nDt - 1))
            t = work.tile([P, H], f32, name=f"al{li}_{e}", tag=f"al{li}_{e}")
            al_t.append(t)
            nc.vector.tensor_tensor(out=t[:], in0=ex_t[e][:], in1=ps[:],
                                    op=Alu.mult)

        # ---- 7. aggregation (head mean folded into psum / block sums)
        if li == 3:
            # lhsT = (Zagg * alpha_k)  [Ep, 8], rhs = wide f32r h chunks
            cch = _nchunks(C, 512)
            tags = ["ps_aggA", "ps_aggB", "ps_aggC"]
            ps_list = [psum.tile([P, c1 - c0], f32, name=tags[i], tag=tags[i],
                                 bufs=1)
                       for i, (c0, c1) in enumerate(cch)]
            for k in range(H):
                for e in range(nE):
                    za = work.tile([P, n_agg], f32r, name="za", tag="za",
                                   bufs=3)
                    nc.vector.tensor_scalar_mul(out=za[:],
                                                in0=pv(rg, "Zagg3", e),
                                                scalar1=al_t[e][:, k:k + 1])
                    first = (k == 0 and e == 0)
                    last = (k == H - 1 and e == nE - 1)
                    for (c0, c1), ps in zip(cch, ps_list):
                        nc.tensor.matmul(out=ps[:n_agg, :],
                                         lhsT=za[:],
                                         rhs=hs(e, k * C + c0, k * C + c1),
                                         start=first, stop=last)
            out_writer(0, n_agg, ps_list, cch)
        else:
            # scale h by alpha in place (per-head broadcast), then matmul
            # 3 heads per instruction; head-mean = sum of the 6 psum blocks.
            for e in range(nE):
                msg = h_t[e][:, :HC].rearrange("p (h c) -> p h c", h=H)
                alb = al_t[e][:].unsqueeze(2).broadcast_to([P, H, C])
                nc.vector.tensor_tensor(out=msg, in0=msg, in1=alb,
                                        op=Alu.mult)
            G = max(1, 512 // C)
            ngrp = (H + G - 1) // G
            tags = ["ps_aggA", "ps_aggB", "ps_aggC"]
            assert ngrp <= len(tags)
            for (d0, d1) in _nchunks(Dup, P):
                rows = d1 - d0
                grp_heads = [list(range(g * G, min((g + 1) * G, H)))
                             for g in range(ngrp)]
                ps_list = [psum.tile([P, len(gh) * C], f32, name=tags[g],
                                     tag=tags[g], bufs=1)
                           for g, gh in enumerate(grp_heads)]
                for g, gh in enumerate(grp_heads):
                    for e in range(nE):
                        nc.tensor.matmul(
                            out=ps_list[g][:rows, :],
                            lhsT=pv(rg, f"Zdst{li}", e, d0, d1),
                            rhs=hs(e, gh[0] * C, (gh[-1] + 1) * C),
                            start=(e == 0), stop=(e == nE - 1))
                blocks = []
                for g, gh in enumerate(grp_heads):
                    for j in range(len(gh)):
                        blocks.append(ps_list[g][:rows, j * C:(j + 1) * C])
                acc = work.tile([P, C], f32, name="accsum", tag="accsum",
                                bufs=2)
                nc.vector.tensor_copy(out=acc[:rows], in_=blocks[0])
                for blk in blocks[1:]:
                    nc.vector.tensor_tensor(out=acc[:rows], in0=acc[:rows],
                                            in1=blk, op=Alu.add)
                out_writer(d0 // P, rows, acc, None)

    def xe_gather(pools, li, lay, X_tiles, Cprev, rg):
        """XE^T [Cprev-tiles of 128, Ep] = X^T routed to edges via Gsrc."""
        work, psum = pools
        Ep, Sp = lay["Ep"], lay["Sp"]
        nS = Sp // P
        XET = []
        for m in range(Cprev // P):
            ps = psum.tile([P, Ep], f32, name="ps_xe", tag="ps_small", bufs=2)
            for s in range(nS):
                nc.tensor.matmul(out=ps[:],
                                 lhsT=X_tiles[s][:, m * P:(m + 1) * P],
                                 rhs=pv(rg, f"Gsrc{li}", s),
                                 start=(s == 0), stop=(s == nS - 1))
            t = work.tile([P, Ep], f32r, name=f"XET{li}_{m}",
                          tag=f"XET{li}_{m}")
            nc.vector.tensor_copy(out=t[:], in_=ps[:])
            XET.append(t)
        return XET

    with tile.TileContext(nc) as tc:
        with tc.tile_pool(name="carry", bufs=1) as carry, \
             tc.tile_pool(name="psum", bufs=1, space="PSUM") as psum:
            ident = carry.tile([P, P], f32, name="ident", tag="ident")
            make_identity(nc, ident[:])

            # all pack images load up-front, one big DMA each
            for nm, p in packs.items():
                dt = f32r if nm.endswith("r") else f32
                t = carry.tile([P, p.cols], dt, name=f"pk_{nm}",
                               tag=f"pk_{nm}")
                nc.sync.dma_start(out=t[:], in_=din[nm][:, :])
                ptile[nm] = t

            # carried node-major activations (f32r: feed xe_gather matmuls)
            X2_t = [carry.tile([P, C_out[0]], f32r, name=f"X2_{i}",
                               tag=f"X2_{i}") for i in range(l2["Sp"] // P)]
            X3_t = [carry.tile([P, C_out[1]], f32r, name=f"X3_{i}",
                               tag=f"X3_{i}") for i in range(l3["Sp"] // P)]
            # ---------------- layer 1
            with tc.tile_pool(name="l1", bufs=1) as w1:
                XE1T_t = [pv("g1r", "XE1T", k)
                          for k in range(_pad(dims[0]) // P)]

                def w1_out(dc, rows, acc, cch):
                    nc.vector.scalar_tensor_tensor(
                        out=X2_t[dc][:rows, :], in0=acc[:rows],
                        scalar=1.0 / H,
                        in1=pv("g1f", "B1", 0, 0, C_out[0])[:rows, :],
                        op0=Alu.mult, op1=Alu.add)
                gat_layer((w1, psum), 1, l1, XE1T_t, "g1r", "g1f", w1_out)

            # ---------------- layer 2
            with tc.tile_pool(name="l2", bufs=1) as w2:
                XE2T_t = xe_gather((w2, psum), 2, l2, X2_t, _pad(C_out[0]),
                                   "g2r")

                def w2_out(dc, rows, acc, cch):
                    nc.vector.scalar_tensor_tensor(
                        out=X3_t[dc][:rows, :], in0=acc[:rows],
                        scalar=1.0 / H,
                        in1=pv("g2f", "B2", 0, 0, C_out[1])[:rows, :],
                        op0=Alu.mult, op1=Alu.add)
                gat_layer((w2, psum), 2, l2, XE2T_t, "g2r", "g2f", w2_out)

            # ---------------- layer 3 (+ residual, output)
            with tc.tile_pool(name="l3", bufs=1) as w3:
                XE3T_t = xe_gather((w3, psum), 3, l3, X3_t, _pad(C_out[1]),
                                   "g3r")
                out_f = w3.tile([8, dims[3]], f32, name="out_f", tag="out_f")

                def w3_out(dc, rows, ps_list, cch):
                    for (c0, c1), ps in zip(cch, ps_list):
                        nc.vector.scalar_tensor_tensor(
                            out=out_f[:rows, c0:c1], in0=ps[:rows, :],
                            scalar=1.0 / H,
                            in1=pv("g3f", "B3", 0, c0, c1)[:rows, :],
                            op0=Alu.mult, op1=Alu.add)
                        nc.vector.tensor_tensor(
                            out=out_f[:rows, c0:c1], in0=out_f[:rows, c0:c1],
                            in1=pv("g3f", "XR", 0, c0, c1)[:rows, :],
                            op=Alu.add)
                gat_layer((w3, psum), 3, l3, XE3T_t, "g3r", "g3f", w3_out)
                nc.sync.dma_start(out=dout[:, :], in_=out_f[:])

    nc.finalize()
    return nc


def kernel(**inputs):
    global LAST_RESULT
    x = inputs["x"]
    edge_index = inputs["edge_index"]
    ptr = inputs["ptr"]
    consts, packs, layers, dims = _host_prep(x, edge_index, ptr, inputs)
    nc = _build_program(packs, layers, dims)

    from concourse.bass_utils import run_bass_kernel_spmd
    in_maps = [consts for _ in range(CORES)]
    res = run_bass_kernel_spmd(nc, in_maps, list(range(CORES)), trace=TRACE)
    LAST_RESULT = res
    return np.asarray(res.results[0]["out"], np.float32)


# revision 11
# speedup vs baseline: 2.2148x; 1.1225x over previous
"""Trainium2 Bass kernel for nn_GAT_15547781612261.

3-layer GATConv (6 heads, concat=False) over an 8192-node / 40960-edge graph
(incl. self loops), with residual, returning final[ptr[1:]-1] -> [8, 1028].

Strategy: only the 8 output rows are needed, so the computation is exactly the
3-hop in-neighborhood of those rows (~500 nodes / ~650 edges at layer 1).  The
host does the integer-only graph slicing and builds 0/1 routing matrices; the
device performs every floating-point operation:

  * per-edge features h_g = x[src_e] @ [W | W@a_src | W@a_dst]  (one matmul)
  * per-dst attention terms gathered via routing matmuls (Gself / ZdstTu)
  * leaky-relu -> clamp -> exp on the edge logits (segment softmax without
    max-subtraction; exact because softmax is shift-invariant and the clamp
    at 80 never binds for sane data)
  * segment sums (softmax denominator and message aggregation) via matmuls
    against the 0/1 dst-routing matrix, with heads accumulated into the
    same PSUM banks so the head-mean is nearly free

Precision split: the bulky matmuls (features, messages) run in float32r
(fast 1-cycle/row PE mode, ~1e-4 rounding); the softmax statistics path
(ed/z/rz gathers) stays in exact fp32 so attention ratios keep full
precision.  Constants are packed into six [128, N] images so each loads
with a single large DMA.

All 8 NeuronCores run the identical program (the pruned problem is far below
one core's roofline; replication avoids collective latency).  Core 0's output
is returned.
"""

import numpy as np

P = 128
H = 6
N_NODES = 8192
CORES = 8

# test harness hooks
TRACE = False
LAST_RESULT = None


def _pad(n, m=P):
    return ((n + m - 1) // m) * m


# ----------------------------------------------------------------------------
# host-side graph slicing (integer work only)
# ----------------------------------------------------------------------------

def _slice_layer(dst_unique, src_all, dst_all):
    """Edges into dst_unique; local indices; self-loop edge of each dst."""
    mask = np.isin(dst_all, dst_unique)
    e_src = src_all[mask]
    e_dst = dst_all[mask]
    src_nodes = np.unique(e_src)
    esl = np.searchsorted(src_nodes, e_src)
    edl = np.searchsorted(dst_unique, e_dst)
    order = np.argsort(edl, kind="stable")
    esl, edl = esl[order], edl[order]
    is_self = e_src[order] == e_dst[order]
    self_edge = np.full(len(dst_unique), -1, np.int64)
    for e_i in np.flatnonzero(is_self):
        if self_edge[edl[e_i]] < 0:
            self_edge[edl[e_i]] = e_i
    assert (self_edge >= 0).all(), "self loop missing for some dst"
    return src_nodes, esl, edl, self_edge


def _routing(esl, edl, self_edge, n_src, n_dst, agg_cols=None):
    """Build 0/1 routing matrices (fp32) for one layer."""
    E = len(esl)
    Ep = _pad(E)
    Sp = _pad(n_src)
    Dup = _pad(n_dst)
    Zdst = np.zeros((Ep, Dup), np.float32)
    Zdst[np.arange(E), edl] = 1.0
    ZdstTu = np.zeros((Dup, Ep), np.float32)
    ZdstTu[edl, np.arange(E)] = 1.0
    Gself = np.zeros((Ep, Dup), np.float32)
    Gself[self_edge, np.arange(n_dst)] = 1.0
    Gsrc = np.zeros((Sp, Ep), np.float32)
    Gsrc[esl, np.arange(E)] = 1.0
    if agg_cols is None:
        Zagg = Zdst
        n_agg = n_dst
    else:
        n_agg = len(agg_cols)
        Zagg = np.zeros((Ep, n_agg), np.float32)
        for col, d in enumerate(agg_cols):
            Zagg[np.arange(E)[edl == d], col] = 1.0
    return dict(E=E, Ep=Ep, Sp=Sp, Du=n_dst, Dup=Dup, n_agg=n_agg,
                Zdst=Zdst, ZdstTu=ZdstTu, Gself=Gself, Gsrc=Gsrc, Zagg=Zagg)


def _fold_weights(W, a_src, a_dst, cinp):
    """[W | W_k @ as_k | W_k @ ad_k], zero-padded to cinp rows."""
    W = np.asarray(W, np.float32)
    a_src = np.asarray(a_src, np.float32)
    a_dst = np.asarray(a_dst, np.float32)
    Cin = W.shape[0]
    C = a_src.shape[1]
    Wh = W.reshape(Cin, H, C)
    Was = np.einsum('ihc,hc->ih', Wh, a_src)
    Wad = np.einsum('ihc,hc->ih', Wh, a_dst)
    Waug = np.concatenate([W, Was, Wad], axis=1)
    out = np.zeros((cinp, Waug.shape[1]), np.float32)
    out[:Cin] = Waug
    return np.ascontiguousarray(out)


class _Pack:
    """Stacks [t*128, C] (or [rows<=128, C]) fp32 arrays into one [128, N]
    image loaded with a single DMA; records per-block column offsets."""

    def __init__(self, name):
        self.name = name
        self.cols = 0
        self.blocks = {}     # key -> (offset, block_cols, n_tiles)
        self.chunks = []

    def add(self, key, arr):
        r, c = arr.shape
        if r <= P:
            tiles = [np.vstack([arr, np.zeros((P - r, c), np.float32)])
                     if r < P else arr]
        else:
            assert r % P == 0
            tiles = [arr[i * P:(i + 1) * P] for i in range(r // P)]
        self.blocks[key] = (self.cols, c, len(tiles))
        for t in tiles:
            self.chunks.append(np.ascontiguousarray(t, np.float32))
            self.cols += c

    def image(self):
        return np.ascontiguousarray(np.concatenate(self.chunks, axis=1))


def _host_prep(x, edge_index, ptr, params):
    x = np.ascontiguousarray(np.asarray(x, np.float32))
    ei = np.asarray(edge_index, np.int64)
    ptr = np.asarray(ptr, np.int64)
    loops = np.arange(N_NODES, dtype=np.int64)
    src_all = np.concatenate([ei[0], loops])
    dst_all = np.concatenate([ei[1], loops])
    R = (ptr[1:] - 1) % N_NODES

    D3u = np.unique(R)
    S3, es3, ed3, se3 = _slice_layer(D3u, src_all, dst_all)
    S2, es2, ed2, se2 = _slice_layer(S3, src_all, dst_all)
    S1, es1, ed1, se1 = _slice_layer(S2, src_all, dst_all)

    l3 = _routing(es3, ed3, se3, len(S3), len(D3u),
                  agg_cols=np.searchsorted(D3u, R))
    l2 = _routing(es2, ed2, se2, len(S2), len(S3))
    l1 = _routing(es1, ed1, se1, len(S1), len(S2))

    dims = [x.shape[1]] + [params[f'as{i}'].shape[1] for i in (1, 2, 3)]

    # layer-1 edge-major routed input: XE1T[:, e] = x[src_global(e)]
    XE1T = np.zeros((_pad(dims[0]), l1["Ep"]), np.float32)
    XE1T[:dims[0], :l1["E"]] = x[S1[es1]].T

    def bias_img(li, rows):
        b = np.asarray(params[f'b{li}'], np.float32)
        return np.ascontiguousarray(
            np.broadcast_to(b[None, :], (rows, len(b))).copy())

    g1r = _Pack("g1r")
    g1r.add("XE1T", XE1T)
    g1r.add("W1", _fold_weights(params['W1'], params['as1'], params['ad1'],
                                _pad(dims[0])))
    g1r.add("Zdst1", l1["Zdst"])
    g1f = _Pack("g1f")
    g1f.add("Gself1", l1["Gself"])
    g1f.add("ZdstTu1", l1["ZdstTu"])
    g1f.add("B1", bias_img(1, P))

    g2r = _Pack("g2r")
    g2r.add("W2", _fold_weights(params['W2'], params['as2'], params['ad2'],
                                _pad(dims[1])))
    g2r.add("Gsrc2", l2["Gsrc"])
    g2r.add("Zdst2", l2["Zdst"])
    g2f = _Pack("g2f")
    g2f.add("Gself2", l2["Gself"])
    g2f.add("ZdstTu2", l2["ZdstTu"])
    g2f.add("B2", bias_img(2, P))

    g3r = _Pack("g3r")
    g3r.add("W3", _fold_weights(params['W3'], params['as3'], params['ad3'],
                                _pad(dims[2])))
    g3r.add("Gsrc3", l3["Gsrc"])
    g3r.add("Zagg3", l3["Zagg"])
    g3f = _Pack("g3f")
    g3f.add("Gself3", l3["Gself"])
    g3f.add("ZdstTu3", l3["ZdstTu"])
    g3f.add("Zdst3", l3["Zdst"])
    g3f.add("B3", bias_img(3, 8))
    g3f.add("XR", np.ascontiguousarray(x[R]))

    packs = dict(g1r=g1r, g1f=g1f, g2r=g2r, g2f=g2f, g3r=g3r, g3f=g3f)
    consts = {nm: p.image() for nm, p in packs.items()}
    return consts, packs, (l1, l2, l3), dims


# ----------------------------------------------------------------------------
# device program
# ----------------------------------------------------------------------------

def _nchunks(total, step):
    out = []
    o = 0
    while o < total:
        out.append((o, min(o + step, total)))
        o += step
    return out


def _build_program(packs, layers, dims):
    import concourse.bacc as bacc
    import concourse.tile as tile
    from concourse import mybir
    from concourse.masks import make_identity

    f32 = mybir.dt.float32
    f32r = mybir.dt.float32r
    Alu = mybir.AluOpType
    Act = mybir.ActivationFunctionType

    l1, l2, l3 = layers
    slopes = [0.2, 0.2, 0.0]
    C_out = [dims[1], dims[2], dims[3]]

    nc = bacc.Bacc("TRN2", target_bir_lowering=False)

    din = {}
    for nm, p in packs.items():
        dt = f32r if nm.endswith("r") else f32
        din[nm] = nc.dram_tensor(nm, [P, p.cols], dt, kind="ExternalInput")
    dout = nc.dram_tensor("out", [8, dims[3]], f32, kind="ExternalOutput")

    # pack sbuf tiles, filled inside the TileContext
    ptile = {}

    def pv(grp, key, t=0, c0=None, c1=None):
        """View of K-tile `t` of block `key` in pack `grp`, cols [c0, c1)."""
        off, c, _ntl = packs[grp].blocks[key]
        lo = off + t * c + (c0 or 0)
        hi = off + t * c + (c1 if c1 is not None else c)
        return ptile[grp][:, lo:hi]

    def gat_layer(pools, li, lay, XET, rg, fg, out_writer):
        """Emit one GAT layer.
        XET: list of [128, Ep] f32r APs (K-tiles of edge-major input).
        rg/fg: pack-group names for f32r / f32 constants.
        out_writer: (dchunk, rows, acc_or_pslist, cch) -> writes node rows."""
        work, psum = pools
        C = C_out[li - 1]
        HC = H * C
        HCw = HC + 2 * H
        Ep, Du, Dup, n_agg = lay["Ep"], lay["Du"], lay["Dup"], lay["n_agg"]
        nE = Ep // P
        nK = len(XET)
        nDt = Dup // P
        slope = slopes[li - 1]

        # ---- 1. per-edge features h_g = XE^T.T @ W_aug   [Ep, HCw] (f32r)
        h_t = []
        for e in range(nE):
            t = work.tile([P, HCw], f32r, name=f"hg{li}_{e}", tag=f"hg{li}_{e}")
            h_t.append(t)
            for (n0, n1) in _nchunks(HCw, 512):
                ps = psum.tile([P, n1 - n0], f32, name="ps_h", tag="ps_h",
                               bufs=2)
                for k in range(nK):
                    nc.tensor.matmul(
                        out=ps[:],
                        lhsT=XET[k][:, e * P:(e + 1) * P],
                        rhs=pv(rg, f"W{li}", k, n0, n1),
                        start=(k == 0), stop=(k == nK - 1))
                if (e + n0 // 512) % 2 == 0:
                    nc.vector.tensor_copy(out=t[:, n0:n1], in_=ps[:])
                else:
                    nc.scalar.copy(out=t[:, n0:n1], in_=ps[:])

        def hs(e, c0, c1, as_f32=False):
            ap = h_t[e][:, c0:c1]
            return ap.bitcast(f32) if as_f32 else ap

        # ---- 2. ed at dst nodes: ed_node[d] = h_g[self_edge(d), ed-cols]
        edn_t = []
        for d in range(nDt):
            t = work.tile([P, H], f32, name=f"edn{li}_{d}", tag=f"edn{li}_{d}")
            edn_t.append(t)
        for (d0, d1) in _nchunks(Dup, P):
            ps = psum.tile([P, H], f32, name="ps_edn", tag="ps_small", bufs=2)
            for e in range(nE):
                nc.tensor.matmul(
                    out=ps[:],
                    lhsT=pv(fg, f"Gself{li}", e, d0, d1),
                    rhs=hs(e, HC + H, HC + 2 * H, True),
                    start=(e == 0), stop=(e == nE - 1))
            nc.vector.tensor_copy(out=edn_t[d0 // P][:], in_=ps[:])

        # ---- 3. per-edge logits -> ex = exp(clamp(lrelu(es + ed_g)))
        ex_t = []
        for e in range(nE):
            ps = psum.tile([P, H], f32, name="ps_edg", tag="ps_small", bufs=2)
            for d in range(nDt):
                nc.tensor.matmul(
                    out=ps[:],
                    lhsT=pv(fg, f"ZdstTu{li}", d, e * P, (e + 1) * P),
                    rhs=edn_t[d][:],
                    start=(d == 0), stop=(d == nDt - 1))
            t = work.tile([P, H], f32, name=f"ex{li}_{e}", tag=f"ex{li}_{e}")
            ex_t.append(t)
            nc.vector.tensor_tensor(out=t[:], in0=hs(e, HC, HC + H, True),
                                    in1=ps[:], op=Alu.add)
            # leaky relu: max(x, slope*x)  (slope in [0, 1])
            nc.vector.scalar_tensor_tensor(out=t[:], in0=t[:],
                                           scalar=float(slope), in1=t[:],
                                           op0=Alu.mult, op1=Alu.max)
            nc.vector.tensor_scalar_min(out=t[:], in0=t[:], scalar1=80.0)
            nc.scalar.activation(out=t[:], in_=t[:], func=Act.Exp)

        # ---- 4. softmax denominators z[h, d] then rz = 1/max(z, tiny)
        zp = psum.tile([H, Dup], f32, name="ps_z", tag="ps_small", bufs=2)
        for e in range(nE):
            rhs = (pv(fg, "Zdst3", e) if li == 3
                   else pv(rg, f"Zdst{li}", e).bitcast(f32))
            nc.tensor.matmul(out=zp[:], lhsT=ex_t[e][:], rhs=rhs,
                             start=(e == 0), stop=(e == nE - 1))
        rz = work.tile([H, Dup], f32, name=f"rz{li}", tag=f"rz{li}")
        nc.vector.tensor_scalar_max(out=rz[:], in0=zp[:], scalar1=1e-30)
        nc.vector.reciprocal(out=rz[:], in_=rz[:])

        # ---- 5. rz transposed to node-major [Dup, H]
        rzT_t = []
        for d in range(nDt):
            t = work.tile([P, H], f32, name=f"rzT{li}_{d}", tag=f"rzT{li}_{d}")
            rzT_t.append(t)
        for (d0, d1) in _nchunks(Dup, P):
            ps = psum.tile([P, H], f32, name="ps_rzT", tag="ps_small", bufs=2)
            nc.tensor.transpose(out=ps[:], in_=rz[:, d0:d1],
                                identity=ident[:H, :H])
            nc.vector.tensor_copy(out=rzT_t[d0 // P][:], in_=ps[:])

        # ---- 6. alpha = ex * rz[dst_e]
        al_t = []
        for e in range(nE):
            ps = psum.tile([P, H], f32, name="ps_rzg", tag="ps_small", bufs=2)
            for d in range(nDt):
                nc.tensor.matmul(
                    out=ps[:],
                    lhsT=pv(fg, f"ZdstTu{li}", d, e * P, (e + 1) * P),
                    rhs=rzT_t[d][:],
                    start=(d == 0), stop=(d == nDt - 1))
            t = work.tile([P, H], f32, name=f"al{li}_{e}", tag=f"al{li}_{e}")
            al_t.append(t)
            nc.vector.tensor_tensor(out=t[:], in0=ex_t[e][:], in1=ps[:],
                                    op=Alu.mult)

        # ---- 7. aggregation (head mean folded into psum / block sums)
        if li == 3:
            # lhsT = (Zagg * alpha_k)  [Ep, 8], rhs = wide f32r h chunks
            cch = _nchunks(C, 512)
            tags = ["ps_aggA", "ps_aggB", "ps_aggC"]
            ps_list = [psum.tile([P, c1 - c0], f32, name=tags[i], tag=tags[i],
                                 bufs=1)
                       for i, (c0, c1) in enumerate(cch)]
            for k in range(H):
                for e in range(nE):
                    za = work.tile([P, n_agg], f32r, name="za", tag="za",
                                   bufs=3)
                    nc.vector.tensor_scalar_mul(out=za[:],
                                                in0=pv(rg, "Zagg3", e),
                                                scalar1=al_t[e][:, k:k + 1])
                    first = (k == 0 and e == 0)
                    last = (k == H - 1 and e == nE - 1)
                    for (c0, c1), ps in zip(cch, ps_list):
                        nc.tensor.matmul(out=ps[:n_agg, :],
                                         lhsT=za[:],
                                         rhs=hs(e, k * C + c0, k * C + c1),
                                         start=first, stop=last)
            out_writer(0, n_agg, ps_list, cch)
        else:
            # scale h by alpha in place (per-head broadcast), then matmul
            # 3 heads per instruction; head-mean = sum of the 6 psum blocks.
            for e in range(nE):
                msg = h_t[e][:, :HC].rearrange("p (h c) -> p h c", h=H)
                alb = al_t[e][:].unsqueeze(2).broadcast_to([P, H, C])
                nc.vector.tensor_tensor(out=msg, in0=msg, in1=alb,
                                        op=Alu.mult)
            G = max(1, 512 // C)
            ngrp = (H + G - 1) // G
            tags = ["ps_aggA", "ps_aggB", "ps_aggC"]
            assert ngrp <= len(tags)
            for (d0, d1) in _nchunks(Dup, P):
                rows = d1 - d0
                grp_heads = [list(range(g * G, min((g + 1) * G, H)))
                             for g in range(ngrp)]
                ps_list = [psum.tile([P, len(gh) * C], f32, name=tags[g],
                                     tag=tags[g], bufs=1)
                           for g, gh in enumerate(grp_heads)]
                for g, gh in enumerate(grp_heads):
                    for e in range(nE):
                        nc.tensor.matmul(
                            out=ps_list[g][:rows, :],
                            lhsT=pv(rg, f"Zdst{li}", e, d0, d1),
                            rhs=hs(e, gh[0] * C, (gh[-1] + 1) * C),
                            start=(e == 0), stop=(e == nE - 1))
                blocks = []
                for g, gh in enumerate(grp_heads):
                    for j in range(len(gh)):
                        blocks.append(ps_list[g][:rows, j * C:(j + 1) * C])
                acc = work.tile([P, C], f32, name="accsum", tag="accsum",
                                bufs=2)
                nc.vector.tensor_copy(out=acc[:rows], in_=blocks[0])
                for blk in blocks[1:]:
                    nc.vector.tensor_tensor(out=acc[:rows], in0=acc[:rows],
                                            in1=blk, op=Alu.add)
                out_writer(d0 // P, rows, acc, None)

    def xe_gather(pools, li, lay, X_tiles, Cprev, rg):
        """XE^T [Cprev-tiles of 128, Ep] = X^T routed to edges via Gsrc."""
        work, psum = pools
        Ep, Sp = lay["Ep"], lay["Sp"]
        nS = Sp // P
        XET = []
        for m in range(Cprev // P):
            ps = psum.tile([P, Ep], f32, name="ps_xe", tag="ps_small", bufs=2)
            for s in range(nS):
                nc.tensor.matmul(out=ps[:],
                                 lhsT=X_tiles[s][:, m * P:(m + 1) * P],
                                 rhs=pv(rg, f"Gsrc{li}", s),
                                 start=(s == 0), stop=(s == nS - 1))
            t = work.tile([P, Ep], f32r, name=f"XET{li}_{m}",
                          tag=f"XET{li}_{m}")
            nc.vector.tensor_copy(out=t[:], in_=ps[:])
            XET.append(t)
        return XET

    with tile.TileContext(nc) as tc:
        with tc.tile_pool(name="carry", bufs=1) as carry, \
             tc.tile_pool(name="psum", bufs=1, space="PSUM") as psum:
            ident = carry.tile([P, P], f32, name="ident", tag="ident")
            make_identity(nc, ident[:])

            # pack images load in ~2MB column chunks (subtile deps let
            # consumers start as soon as their columns land)
            for nm, p in packs.items():
                dt = f32r if nm.endswith("r") else f32
                t = carry.tile([P, p.cols], dt, name=f"pk_{nm}",
                               tag=f"pk_{nm}")
                for (c0, c1) in _nchunks(p.cols, 4096):
                    nc.sync.dma_start(out=t[:, c0:c1], in_=din[nm][:, c0:c1])
                ptile[nm] = t

            # carried node-major activations (f32r: feed xe_gather matmuls)
            X2_t = [carry.tile([P, C_out[0]], f32r, name=f"X2_{i}",
                               tag=f"X2_{i}") for i in range(l2["Sp"] // P)]
            X3_t = [carry.tile([P, C_out[1]], f32r, name=f"X3_{i}",
                               tag=f"X3_{i}") for i in range(l3["Sp"] // P)]
            # ---------------- layer 1
            with tc.tile_pool(name="l1", bufs=1) as w1:
                XE1T_t = [pv("g1r", "XE1T", k)
                          for k in range(_pad(dims[0]) // P)]

                def w1_out(dc, rows, acc, cch):
                    nc.vector.scalar_tensor_tensor(
                        out=X2_t[dc][:rows, :], in0=acc[:rows],
                        scalar=1.0 / H,
                        in1=pv("g1f", "B1", 0, 0, C_out[0])[:rows, :],
                        op0=Alu.mult, op1=Alu.add)
                gat_layer((w1, psum), 1, l1, XE1T_t, "g1r", "g1f", w1_out)

            # ---------------- layer 2
            with tc.tile_pool(name="l2", bufs=1) as w2:
                XE2T_t = xe_gather((w2, psum), 2, l2, X2_t, _pad(C_out[0]),
                                   "g2r")

                def w2_out(dc, rows, acc, cch):
                    nc.vector.scalar_tensor_tensor(
                        out=X3_t[dc][:rows, :], in0=acc[:rows],
                        scalar=1.0 / H,
                        in1=pv("g2f", "B2", 0, 0, C_out[1])[:rows, :],
                        op0=Alu.mult, op1=Alu.add)
                gat_layer((w2, psum), 2, l2, XE2T_t, "g2r", "g2f", w2_out)

            # ---------------- layer 3 (+ residual, output)
            with tc.tile_pool(name="l3", bufs=1) as w3:
                XE3T_t = xe_gather((w3, psum), 3, l3, X3_t, _pad(C_out[1]),
                                   "g3r")
                out_f = w3.tile([8, dims[3]], f32, name="out_f", tag="out_f")

                def w3_out(dc, rows, ps_list, cch):
                    for (c0, c1), ps in zip(cch, ps_list):
                        nc.vector.scalar_tensor_tensor(
                            out=out_f[:rows, c0:c1], in0=ps[:rows, :],
                            scalar=1.0 / H,
                            in1=pv("g3f", "B3", 0, c0, c1)[:rows, :],
                            op0=Alu.mult, op1=Alu.add)
                        nc.vector.tensor_tensor(
                            out=out_f[:rows, c0:c1], in0=out_f[:rows, c0:c1],
                            in1=pv("g3f", "XR", 0, c0, c1)[:rows, :],
                            op=Alu.add)
                gat_layer((w3, psum), 3, l3, XE3T_t, "g3r", "g3f", w3_out)
                nc.sync.dma_start(out=dout[:, :], in_=out_f[:])

    nc.finalize()
    return nc


def kernel(**inputs):
    global LAST_RESULT
    x = inputs["x"]
    edge_index = inputs["edge_index"]
    ptr = inputs["ptr"]
    consts, packs, layers, dims = _host_prep(x, edge_index, ptr, inputs)
    nc = _build_program(packs, layers, dims)

    from concourse.bass_utils import run_bass_kernel_spmd
    in_maps = [consts for _ in range(CORES)]
    res = run_bass_kernel_spmd(nc, in_maps, list(range(CORES)), trace=TRACE)
    LAST_RESULT = res
    return np.asarray(res.results[0]["out"], np.float32)
